# revision 7
# baseline (speedup 1.0000x reference)
"""Causal self-attention on 8 trn2 NeuronCores.

Sharding: data-parallel over batch (2) x tensor-parallel over heads (4/core).
Core c handles batch c//4, heads (c%4)*4 .. (c%4)*4+4.  Each core computes
QKV projection for its heads, causal attention, and a partial c_proj
(y_local @ w_proj[local rows]); the host sums the 4 partials per batch and
adds nothing else (b_proj is folded in as b_proj/4 on every core).

Device kernel notes:
- All matmuls run as float32r (full fp32 data, PE reads reduced mantissa)
  which streams at 1 cycle/row for free-dim >= 256 (4x faster than fp32).
- Attention uses the S^T = K Q^T orientation so the softmax reduction is a
  matmul: V is augmented with a ones column, so A@V also yields the softmax
  denominator, and the normalization happens as a per-query (free-dim)
  scale after a gpsimd partition_broadcast.
- exp runs without max-subtraction (scores are bounded ~|3| for this
  problem family; exp of a masked -1e9 underflows to 0 exactly as the
  reference's softmax does).  Causality is handled by tile skipping plus a
  single static 128x128 tril mask on diagonal band tiles.
"""

import os
import sys

for p in ("/root/.axon_site", "/root/.axon_site/_ro/trn_rl_repo", "/root/.axon_site/_ro/pypackages", "/opt/trn_rl_repo"):
    if os.path.isdir(p) and p not in sys.path:
        sys.path.append(p)

import numpy as np

import concourse.bacc as bacc
import concourse.mybir as mybir
import concourse.tile as tile
from concourse.bass_utils import run_bass_kernel_spmd

F32 = mybir.dt.float32
F32R = mybir.dt.float32r
Exp = mybir.ActivationFunctionType.Exp
MULT = mybir.AluOpType.mult
ADD = mybir.AluOpType.add

T = 2048            # sequence length (per batch)
C = 1024            # embedding dim
NHL = 4             # heads per core
HD = 64             # head dim
FL = NHL * HD       # local features (256)
CK = C // 128       # contraction chunks (8)
NQG = T // 512      # query groups of 512 (4)
NKT = T // 128      # key tiles of 128 (16)
NTT = T // 128      # token tiles of 128 (16)

_CACHE = {}
LAST_RESULTS = None


def _build():
    nc = bacc.Bacc("TRN2", target_bir_lowering=False, debug=False, num_devices=8)

    xT = nc.dram_tensor("xT", [C, T], F32, kind="ExternalInput").ap()
    wq = nc.dram_tensor("wq", [C, FL], F32, kind="ExternalInput").ap()
    wk = nc.dram_tensor("wk", [C, FL], F32, kind="ExternalInput").ap()
    wv = nc.dram_tensor("wv", [C, FL], F32, kind="ExternalInput").ap()
    bq = nc.dram_tensor("bq", [FL, 1], F32, kind="ExternalInput").ap()
    bk = nc.dram_tensor("bk", [FL, 1], F32, kind="ExternalInput").ap()
    bv = nc.dram_tensor("bv", [1, FL], F32, kind="ExternalInput").ap()
    wp = nc.dram_tensor("wp", [FL, C], F32, kind="ExternalInput").ap()
    bp4 = nc.dram_tensor("bp4", [1, C], F32, kind="ExternalInput").ap()
    tril = nc.dram_tensor("tril", [128, 128], F32, kind="ExternalInput").ap()
    vones = nc.dram_tensor("vones", [128, NTT * NHL], F32, kind="ExternalInput").ap()
    out = nc.dram_tensor("out", [T, C], F32, kind="ExternalOutput").ap()

    with tile.TileContext(nc) as tc:
        with (
            tc.tile_pool(name="persist", bufs=1) as pp,
            tc.tile_pool(name="xpool", bufs=1) as xp,
            tc.tile_pool(name="attp", bufs=6) as ap_,
            tc.tile_pool(name="smallp", bufs=3) as sp,
            tc.tile_pool(name="outp", bufs=2) as op_,
            tc.tile_pool(name="proj_ps", bufs=2, space="PSUM") as pps,
            tc.tile_pool(name="s_ps", bufs=4, space="PSUM") as sps,
            tc.tile_pool(name="av_ps", bufs=1, space="PSUM") as avps,
        ):
            # ---- persistent SBUF tensors ----
            xt_sb = xp.tile([128, CK * T], F32R)          # 8 chunks of x^T [128, 2048]
            wq_sb = pp.tile([128, CK * FL], F32R)         # 8 chunks [128, 256]
            wk_sb = pp.tile([128, CK * FL], F32R)
            wv_sb = pp.tile([128, CK * FL], F32R)
            qt_sb = [pp.tile([128, T], F32R, tag=f"qt{p}", name=f"qt{p}") for p in range(2)]
            kt_sb = [pp.tile([128, T], F32R, tag=f"kt{p}", name=f"kt{p}") for p in range(2)]
            v_sb = pp.tile([128, NTT * NHL * (HD + 1)], F32R, tag="v")  # per tile: 4x65
            yt_sb = [pp.tile([128, T], F32R, tag=f"yt{p}", name=f"yt{p}") for p in range(2)]
            wp_sb = pp.tile([128, 2 * C], F32R)
            bq_sb = pp.tile([128, 2], F32, tag="bq")
            bk_sb = pp.tile([128, 2], F32, tag="bk")
            bv_bc = pp.tile([128, FL], F32, tag="bvbc")
            bp_bc = pp.tile([128, C], F32, tag="bpbc")
            tril_sb = pp.tile([128, 128], F32, tag="tril")
            bv_row = pp.tile([1, FL], F32, tag="bvrow")
            bp_row = pp.tile([1, C], F32, tag="bprow")

            # ---- input DMAs ----
            for ck in range(CK):
                nc.sync.dma_start(out=xt_sb[:, ck * T:(ck + 1) * T],
                                  in_=xT[ck * 128:(ck + 1) * 128, :].bitcast(F32R))
                nc.sync.dma_start(out=wq_sb[:, ck * FL:(ck + 1) * FL],
                                  in_=wq[ck * 128:(ck + 1) * 128, :].bitcast(F32R))
                nc.sync.dma_start(out=wk_sb[:, ck * FL:(ck + 1) * FL],
                                  in_=wk[ck * 128:(ck + 1) * 128, :].bitcast(F32R))
                nc.sync.dma_start(out=wv_sb[:, ck * FL:(ck + 1) * FL],
                                  in_=wv[ck * 128:(ck + 1) * 128, :].bitcast(F32R))
            for f in range(2):
                nc.sync.dma_start(out=wp_sb[:, f * C:(f + 1) * C],
                                  in_=wp[f * 128:(f + 1) * 128, :].bitcast(F32R))
                nc.sync.dma_start(out=bq_sb[:, f:f + 1], in_=bq[f * 128:(f + 1) * 128, :])
                nc.sync.dma_start(out=bk_sb[:, f:f + 1], in_=bk[f * 128:(f + 1) * 128, :])
            nc.sync.dma_start(out=tril_sb[:], in_=tril)
            nc.sync.dma_start(out=bv_row[:], in_=bv)
            nc.sync.dma_start(out=bp_row[:], in_=bp4)
            nc.gpsimd.partition_broadcast(bv_bc[:], bv_row[:])
            nc.gpsimd.partition_broadcast(bp_bc[:], bp_row[:])
            # ones column of the augmented V (col 64 of each head block)
            v_ones = v_sb[:].rearrange("p (n c) -> p n c", c=HD + 1)[:, :, HD]
            nc.sync.dma_start(out=v_ones, in_=vones.bitcast(F32R))

            def v_tile(tt):
                return v_sb[:, tt * NHL * (HD + 1):(tt + 1) * NHL * (HD + 1)]

            # ---- per-query-group pipeline: projections then attention ----
            for qg in range(NQG):
                qs = qg * 512
                # K^T and Q^T for this query group, both head pairs
                for w_sb, b_sb, dst in ((wk_sb, bk_sb, kt_sb), (wq_sb, bq_sb, qt_sb)):
                    for p in range(2):
                        ps = pps.tile([128, 512], F32, tag="proj")
                        for ck in range(CK):
                            nc.tensor.matmul(
                                ps[:],
                                w_sb[:, ck * FL + p * 128: ck * FL + (p + 1) * 128],
                                xt_sb[:, ck * T + qs: ck * T + qs + 512],
                                start=(ck == 0), stop=(ck == CK - 1))
                        nc.vector.tensor_scalar_add(dst[p][:, qs:qs + 512], ps[:], b_sb[:, p:p + 1])
                # V for this group's token tiles (natural orientation + bias)
                for tt in range(4 * qg, 4 * qg + 4):
                    ps = pps.tile([128, 512], F32, tag="proj")
                    for ck in range(CK):
                        nc.tensor.matmul(
                            ps[:, 0:FL],
                            xt_sb[:, ck * T + tt * 128: ck * T + (tt + 1) * 128],
                            wv_sb[:, ck * FL:(ck + 1) * FL],
                            start=(ck == 0), stop=(ck == CK - 1))
                    vdst = v_tile(tt).rearrange("p (n c) -> p n c", c=HD + 1)[:, :, 0:HD]
                    nc.vector.tensor_tensor(vdst, ps[:, 0:FL].rearrange("p (n c) -> p n c", c=HD), bv_bc[:].rearrange("p (n c) -> p n c", c=HD), ADD)

                # attention for this query group, per head pair
                for p in range(2):
                    av_ps = [avps.tile([65, 512], F32, tag=f"av{h}", name=f"av{h}") for h in range(2)]
                    for kt in range(4 * qg + 4):
                        d = max(0, (kt - 4 * qg) * 128)  # 0 for full tiles
                        n = 512 - d
                        att = [None, None]
                        s_ps = [sps.tile([128, 512], F32, tag="s", name="s_ps") for _ in range(2)]
                        for h in range(2):
                            nc.tensor.matmul(
                                s_ps[h][:, 0:n],
                                kt_sb[p][h * 64:(h + 1) * 64, kt * 128:(kt + 1) * 128],
                                qt_sb[p][h * 64:(h + 1) * 64, qs + d: qs + 512],
                                start=True, stop=True)
                        for h in range(2):
                            att[h] = ap_.tile([128, 512], F32R, tag="att", name="att")
                            if kt < 4 * qg:  # fully causal-valid tile
                                nc.scalar.activation(att[h][:, 0:512], s_ps[h][:, 0:512], Exp, scale=0.125)
                            else:
                                # diagonal band [d, d+128) gets exp * tril; tail is plain exp
                                if 512 - (d + 128) > 0:
                                    nc.scalar.activation(att[h][:, d + 128:512], s_ps[h][:, 128:n], Exp, scale=0.125)
                                tmp = sp.tile([128, 128], F32, tag="band")
                                nc.scalar.activation(tmp[:], s_ps[h][:, 0:128], Exp, scale=0.125)
                                nc.vector.tensor_tensor(att[h][:, d:d + 128], tmp[:], tril_sb[:], MULT)
                        for h in range(2):
                            nc.tensor.matmul(
                                av_ps[h][:, d:512],
                                v_tile(kt)[:, (2 * p + h) * (HD + 1):(2 * p + h + 1) * (HD + 1)],
                                att[h][:, d:512],
                                start=(kt == 0), stop=(kt == 4 * qg + 3))
                    for h in range(2):
                        dinv = sp.tile([1, 512], F32, tag="dinv")
                        bc = sp.tile([64, 512], F32, tag="bc")
                        nc.vector.reciprocal(dinv[:], av_ps[h][64:65, :])
                        nc.gpsimd.partition_broadcast(bc[:], dinv[:])
                        nc.vector.tensor_tensor(yt_sb[p][h * 64:(h + 1) * 64, qs:qs + 512],
                                                av_ps[h][0:64, :], bc[:], MULT)

            # ---- c_proj partial: out[t, :] = y_local^T.T @ wp + b_proj/4 ----
            for tt in range(NTT):
                ob = op_.tile([128, C], F32, tag="ob")
                for ng in range(2):
                    ps = pps.tile([128, 512], F32, tag="proj")
                    for f in range(2):
                        nc.tensor.matmul(
                            ps[:],
                            yt_sb[f][:, tt * 128:(tt + 1) * 128],
                            wp_sb[:, f * C + ng * 512: f * C + ng * 512 + 512],
                            start=(f == 0), stop=(f == 1))
                    nc.vector.tensor_tensor(ob[:, ng * 512:(ng + 1) * 512], ps[:], bp_bc[:, ng * 512:(ng + 1) * 512], ADD)
                nc.sync.dma_start(out=out[tt * 128:(tt + 1) * 128, :], in_=ob[:])

    nc.compile()
    return nc


def kernel(x, w_attn, b_attn, w_proj, b_proj):
    global LAST_RESULTS
    x = np.asarray(x, dtype=np.float32)
    w_attn = np.asarray(w_attn, dtype=np.float32)
    b_attn = np.asarray(b_attn, dtype=np.float32)
    w_proj = np.asarray(w_proj, dtype=np.float32)
    b_proj = np.asarray(b_proj, dtype=np.float32)
    b, t, c = x.shape
    assert (b, t, c) == (2, T, C)

    if "nc" not in _CACHE:
        _CACHE["nc"] = _build()
    nc = _CACHE["nc"]

    trilm = np.triu(np.ones((128, 128), dtype=np.float32))  # [k, q] orientation: valid iff k <= q
    in_maps = []
    for core in range(8):
        bi, g = divmod(core, 4)
        cs = FL * g  # column/row offset for this core's 4 heads
        in_maps.append({
            "xT": np.ascontiguousarray(x[bi].T),
            "wq": np.ascontiguousarray(w_attn[:, cs:cs + FL]),
            "wk": np.ascontiguousarray(w_attn[:, C + cs:C + cs + FL]),
            "wv": np.ascontiguousarray(w_attn[:, 2 * C + cs:2 * C + cs + FL]),
            "bq": np.ascontiguousarray(b_attn[cs:cs + FL].reshape(FL, 1)),
            "bk": np.ascontiguousarray(b_attn[C + cs:C + cs + FL].reshape(FL, 1)),
            "bv": np.ascontiguousarray(b_attn[2 * C + cs:2 * C + cs + FL].reshape(1, FL)),
            "wp": np.ascontiguousarray(w_proj[cs:cs + FL, :]),
            "bp4": np.ascontiguousarray((b_proj / 4.0).reshape(1, C)),
            "tril": trilm,
            "vones": np.ones((128, NTT * NHL), dtype=np.float32),
        })

    res = run_bass_kernel_spmd(nc, in_maps, core_ids=list(range(8)))
    LAST_RESULTS = res
    # unshard: sum the 4 tensor-parallel partials of each batch element
    y = np.empty((2, T, C), dtype=np.float32)
    for bi in range(2):
        acc = res.results[4 * bi][ "out"].astype(np.float32)
        for g in range(1, 4):
            acc = acc + res.results[4 * bi + g]["out"]
        y[bi] = acc
    return y


# revision 19
# speedup vs baseline: 1.3188x; 1.3188x over previous
"""Causal self-attention on 8 trn2 NeuronCores.

Sharding: data-parallel over batch (2) x tensor-parallel over heads (4/core).
Core c handles batch c//4, heads (c%4)*4 .. (c%4)*4+4.  Each core computes
QKV projection for its heads, causal attention, and a partial c_proj
(y_local @ w_proj[local rows]); the host sums the 4 partials per batch
(b_proj is folded in as b_proj/4 on every core).

Device kernel notes:
- Matmul inputs are bf16 (host-converted); accumulation stays fp32 in PSUM.
  bf16 keeps LDWEIGHTS as a separate, pipelined instruction (fp32/fp32r
  matmuls self-load weights serially, measured +~180ns on every matmul).
- Attention uses the S^T = K Q^T orientation so the softmax reduction is a
  matmul: V is augmented with a ones column, so A@V also yields the softmax
  denominator; normalization is a per-query free-dim scale applied after a
  gpsimd partition_broadcast of reciprocal_approx_fast(denominator).
- exp runs without max-subtraction (scores are bounded ~|3| for this
  problem family; a masked entry's exp(-1e9) underflow to 0 matches the
  reference's softmax exactly).  Causality is tile-skipped; diagonal band
  tiles get one in-place multiply with a static 128x128 triu mask.
"""

import os
import sys

for p in ("/root/.axon_site", "/root/.axon_site/_ro/trn_rl_repo", "/root/.axon_site/_ro/pypackages", "/opt/trn_rl_repo"):
    if os.path.isdir(p) and p not in sys.path:
        sys.path.append(p)

import ml_dtypes
import numpy as np

import concourse.bacc as bacc
import concourse.mybir as mybir
import concourse.tile as tile
from concourse.bass_utils import run_bass_kernel_spmd

F32 = mybir.dt.float32
BF16 = mybir.dt.bfloat16
Exp = mybir.ActivationFunctionType.Exp
MULT = mybir.AluOpType.mult
ADD = mybir.AluOpType.add
BF = ml_dtypes.bfloat16

T = 2048            # sequence length (per batch)
C = 1024            # embedding dim
NHL = 4             # heads per core
HD = 64             # head dim
FL = NHL * HD       # local features (256)
CK = C // 128       # contraction chunks (8)
NQG = T // 512      # query groups of 512 (4)
NTT = T // 128      # token tiles of 128 (16)

_CACHE = {}
LAST_RESULTS = None


def _build():
    nc = bacc.Bacc("TRN2", target_bir_lowering=False, debug=False, num_devices=8)

    xT = nc.dram_tensor("xT", [C, T], BF16, kind="ExternalInput").ap()
    wq = nc.dram_tensor("wq", [C, FL], BF16, kind="ExternalInput").ap()
    wk = nc.dram_tensor("wk", [C, FL], BF16, kind="ExternalInput").ap()
    wv = nc.dram_tensor("wv", [C, FL], BF16, kind="ExternalInput").ap()
    bq = nc.dram_tensor("bq", [FL, 1], F32, kind="ExternalInput").ap()
    bk = nc.dram_tensor("bk", [FL, 1], F32, kind="ExternalInput").ap()
    bv = nc.dram_tensor("bv", [1, FL], F32, kind="ExternalInput").ap()
    wp = nc.dram_tensor("wp", [FL, C], BF16, kind="ExternalInput").ap()
    bpf = nc.dram_tensor("bpf", [1, C], F32, kind="ExternalInput").ap()
    tril = nc.dram_tensor("tril", [128, 128], BF16, kind="ExternalInput").ap()
    vones = nc.dram_tensor("vones", [128, NTT * NHL], BF16, kind="ExternalInput").ap()
    out = nc.dram_tensor("out", [T, C], F32, kind="ExternalOutput").ap()

    with tile.TileContext(nc) as tc:
        with (
            tc.tile_pool(name="persist", bufs=1) as pp,
            tc.tile_pool(name="xpool", bufs=1) as xp,
            tc.tile_pool(name="attp", bufs=12) as ap_,
            tc.tile_pool(name="smallp", bufs=4) as sp,
            tc.tile_pool(name="outp", bufs=2) as op_,
            tc.tile_pool(name="proj_ps", bufs=2, space="PSUM") as pps,
            tc.tile_pool(name="s_ps", bufs=4, space="PSUM") as sps,
            tc.tile_pool(name="av_ps", bufs=1, space="PSUM") as avps,
        ):
            # ---- persistent SBUF tensors ----
            xt_sb = xp.tile([128, CK * T], BF16)          # 8 chunks of x^T [128, 2048]
            wq_sb = pp.tile([128, CK * FL], BF16)         # 8 chunks [128, 256]
            wk_sb = pp.tile([128, CK * FL], BF16)
            wv_sb = pp.tile([128, CK * FL], BF16)
            qt_sb = [pp.tile([128, T], BF16, tag=f"qt{p}", name=f"qt{p}") for p in range(2)]
            kt_sb = [pp.tile([128, T], BF16, tag=f"kt{p}", name=f"kt{p}") for p in range(2)]
            v_sb = pp.tile([128, NTT * NHL * (HD + 1)], BF16, tag="v")  # per tile: 4x65
            yt_sb = [pp.tile([128, T], BF16, tag=f"yt{p}", name=f"yt{p}") for p in range(2)]
            wp_sb = pp.tile([128, 2 * C], BF16)
            bq_sb = pp.tile([128, 2], F32, tag="bq")
            bk_sb = pp.tile([128, 2], F32, tag="bk")
            bv_bc = pp.tile([128, FL], F32, tag="bvbc")
            bp_bc = pp.tile([128, C], F32, tag="bpbc")
            bp_row = pp.tile([1, C], F32, tag="bprowf")
            tril_sb = pp.tile([128, 128], BF16, tag="tril")
            bv_row = pp.tile([1, FL], F32, tag="bvrow")
            vones_sb = pp.tile([128, NTT * NHL], BF16, tag="vones")

            # ---- input DMAs ----
            for ck in range(CK):
                nc.sync.dma_start(out=xt_sb[:, ck * T:(ck + 1) * T],
                                  in_=xT[ck * 128:(ck + 1) * 128, :])
                nc.sync.dma_start(out=wq_sb[:, ck * FL:(ck + 1) * FL],
                                  in_=wq[ck * 128:(ck + 1) * 128, :])
                nc.sync.dma_start(out=wk_sb[:, ck * FL:(ck + 1) * FL],
                                  in_=wk[ck * 128:(ck + 1) * 128, :])
                nc.sync.dma_start(out=wv_sb[:, ck * FL:(ck + 1) * FL],
                                  in_=wv[ck * 128:(ck + 1) * 128, :])
            for f in range(2):
                nc.sync.dma_start(out=wp_sb[:, f * C:(f + 1) * C],
                                  in_=wp[f * 128:(f + 1) * 128, :])
            # single-DMA loads for the per-partition bias columns (two DMAs
            # into adjacent 4B columns risk sub-granule write clobbering)
            nc.sync.dma_start(out=bq_sb[:], in_=bq.rearrange("(f p) o -> p (f o)", f=2))
            nc.sync.dma_start(out=bk_sb[:], in_=bk.rearrange("(f p) o -> p (f o)", f=2))
            nc.sync.dma_start(out=tril_sb[:], in_=tril)
            nc.sync.dma_start(out=bv_row[:], in_=bv)
            nc.sync.dma_start(out=vones_sb[:], in_=vones)
            nc.sync.dma_start(out=bp_row[:], in_=bpf)
            nc.gpsimd.partition_broadcast(bv_bc[:], bv_row[:])
            nc.gpsimd.partition_broadcast(bp_bc[:], bp_row[:])
            # ones column of the augmented V (col 64 of each head block),
            # written by DVE so it serializes with the DVE value writes
            # (a strided 2-byte DMA interleaved with engine writes flaked)
            v_ones = v_sb[:].rearrange("p (n c) -> p n c", c=HD + 1)[:, :, HD]
            nc.vector.tensor_copy(v_ones, vones_sb[:])

            def v_tile(tt):
                return v_sb[:, tt * NHL * (HD + 1):(tt + 1) * NHL * (HD + 1)]

            # ---- per-query-group pipeline: projections then attention ----
            for qg in range(NQG):
                qs = qg * 512
                # K^T and Q^T for this query group, both head pairs
                for w_sb, b_sb, dst in ((wk_sb, bk_sb, kt_sb), (wq_sb, bq_sb, qt_sb)):
                    for p in range(2):
                        ps = pps.tile([128, 512], F32, tag="proj", name="proj_ps")
                        for ck in range(CK):
                            nc.tensor.matmul(
                                ps[:],
                                w_sb[:, ck * FL + p * 128: ck * FL + (p + 1) * 128],
                                xt_sb[:, ck * T + qs: ck * T + qs + 512],
                                start=(ck == 0), stop=(ck == CK - 1))
                        nc.vector.tensor_scalar_add(dst[p][:, qs:qs + 512], ps[:], b_sb[:, p:p + 1])
                # V for this group's token tiles (natural orientation + bias)
                for tt in range(4 * qg, 4 * qg + 4):
                    ps = pps.tile([128, 512], F32, tag="proj", name="proj_ps")
                    for ck in range(CK):
                        nc.tensor.matmul(
                            ps[:, 0:FL],
                            xt_sb[:, ck * T + tt * 128: ck * T + (tt + 1) * 128],
                            wv_sb[:, ck * FL:(ck + 1) * FL],
                            start=(ck == 0), stop=(ck == CK - 1))
                    vdst = v_tile(tt).rearrange("p (n c) -> p n c", c=HD + 1)[:, :, 0:HD]
                    nc.vector.tensor_tensor(vdst, ps[:, 0:FL].rearrange("p (n c) -> p n c", c=HD), bv_bc[:].rearrange("p (n c) -> p n c", c=HD), ADD)

                # attention for this query group, per head pair
                for p in range(2):
                    av_ps = [avps.tile([65, 512], F32, tag=f"av{h}", name=f"av{h}") for h in range(2)]
                    for kt in range(4 * qg + 4):
                        d = max(0, (kt - 4 * qg) * 128)  # 0 for full tiles
                        n = 512 - d
                        att = [None, None]
                        s_ps = [sps.tile([128, 512], F32, tag="s", name="s_ps") for _ in range(2)]
                        for h in range(2):
                            nc.tensor.matmul(
                                s_ps[h][:, 0:n],
                                kt_sb[p][h * 64:(h + 1) * 64, kt * 128:(kt + 1) * 128],
                                qt_sb[p][h * 64:(h + 1) * 64, qs + d: qs + 512],
                                start=True, stop=True)
                        for h in range(2):
                            att[h] = ap_.tile([128, 512], BF16, tag="att", name="att")
                            nc.scalar.activation(att[h][:, d:512], s_ps[h][:, 0:n], Exp, scale=0.125)
                            if kt >= 4 * qg:
                                # in-place causal mask on the diagonal band
                                nc.vector.tensor_tensor(att[h][:, d:d + 128], att[h][:, d:d + 128], tril_sb[:], MULT)
                        for h in range(2):
                            nc.tensor.matmul(
                                av_ps[h][:, d:512],
                                v_tile(kt)[:, (2 * p + h) * (HD + 1):(2 * p + h + 1) * (HD + 1)],
                                att[h][:, d:512],
                                start=(kt == 0), stop=(kt == 4 * qg + 3))
                    for h in range(2):
                        dinv = sp.tile([1, 512], F32, tag="dinv", name="dinv")
                        bc = sp.tile([64, 512], F32, tag="bc", name="bc")
                        dsb = sp.tile([1, 512], F32, tag="dsb", name="dsb")
                        # custom-DVE reciprocal misreads PSUM at partition
                        # offset 64 on HW; stage the row through SBUF
                        nc.vector.tensor_copy(dsb[:], av_ps[h][64:65, :])
                        nc.vector.reciprocal_approx_fast(out=dinv[:], in_=dsb[:])
                        nc.gpsimd.partition_broadcast(bc[:], dinv[:])
                        nc.vector.tensor_tensor(yt_sb[p][h * 64:(h + 1) * 64, qs:qs + 512],
                                                av_ps[h][0:64, :], bc[:], MULT)

                # c_proj partial for this group's token tiles:
                # out[t, :] = y_local^T.T @ wp + b_proj/4
                for tt in range(4 * qg, 4 * qg + 4):
                    ob = op_.tile([128, C], F32, tag="ob", name="ob")
                    for ng in range(2):
                        ps = pps.tile([128, 512], F32, tag="proj", name="proj_ps")
                        for f in range(2):
                            nc.tensor.matmul(
                                ps[:],
                                yt_sb[f][:, tt * 128:(tt + 1) * 128],
                                wp_sb[:, f * C + ng * 512: f * C + ng * 512 + 512],
                                start=(f == 0), stop=(f == 1))
                        nc.vector.tensor_tensor(ob[:, ng * 512:(ng + 1) * 512], ps[:],
                                                bp_bc[:, ng * 512:(ng + 1) * 512], ADD)
                    nc.sync.dma_start(out=out[tt * 128:(tt + 1) * 128, :], in_=ob[:])

    nc.compile()
    return nc


def kernel(x, w_attn, b_attn, w_proj, b_proj):
    global LAST_RESULTS
    x = np.asarray(x, dtype=np.float32)
    w_attn = np.asarray(w_attn, dtype=np.float32)
    b_attn = np.asarray(b_attn, dtype=np.float32)
    w_proj = np.asarray(w_proj, dtype=np.float32)
    b_proj = np.asarray(b_proj, dtype=np.float32)
    b, t, c = x.shape
    assert (b, t, c) == (2, T, C)

    if "nc" not in _CACHE:
        _CACHE["nc"] = _build()
    nc = _CACHE["nc"]

    trilm = np.triu(np.ones((128, 128), dtype=np.float32))  # [k, q]: valid iff k <= q
    in_maps = []
    for core in range(8):
        bi, g = divmod(core, 4)
        cs = FL * g  # column/row offset for this core's 4 heads
        in_maps.append({
            "xT": np.ascontiguousarray(x[bi].T).astype(BF),
            "wq": np.ascontiguousarray(w_attn[:, cs:cs + FL]).astype(BF),
            "wk": np.ascontiguousarray(w_attn[:, C + cs:C + cs + FL]).astype(BF),
            "wv": np.ascontiguousarray(w_attn[:, 2 * C + cs:2 * C + cs + FL]).astype(BF),
            "bq": np.ascontiguousarray(b_attn[cs:cs + FL].reshape(FL, 1)),
            "bk": np.ascontiguousarray(b_attn[C + cs:C + cs + FL].reshape(FL, 1)),
            "bv": np.ascontiguousarray(b_attn[2 * C + cs:2 * C + cs + FL].reshape(1, FL)),
            "wp": np.ascontiguousarray(w_proj[cs:cs + FL, :]).astype(BF),
            "bpf": (b_proj / 4.0).reshape(1, C),
            "tril": trilm.astype(BF),
            "vones": np.ones((128, NTT * NHL), dtype=BF),
        })

    res = run_bass_kernel_spmd(nc, in_maps, core_ids=list(range(8)))
    LAST_RESULTS = res
    # unshard: sum the 4 tensor-parallel partials of each batch element
    y = np.empty((2, T, C), dtype=np.float32)
    for bi in range(2):
        acc = res.results[4 * bi]["out"].astype(np.float32)
        for g in range(1, 4):
            acc = acc + res.results[4 * bi + g]["out"]
        y[bi] = acc
    return y


# revision 20
# speedup vs baseline: 1.3472x; 1.0216x over previous
"""Causal self-attention on 8 trn2 NeuronCores.

Sharding: data-parallel over batch (2) x tensor-parallel over heads (4/core).
Core c handles batch c//4, heads (c%4)*4 .. (c%4)*4+4.  Each core computes
QKV projection for its heads, causal attention, and a partial c_proj
(y_local @ w_proj[local rows]); the host sums the 4 partials per batch
(b_proj is folded in as b_proj/4 on every core).

Device kernel notes:
- Matmul inputs are bf16 (host-converted); accumulation stays fp32 in PSUM.
  bf16 keeps LDWEIGHTS as a separate, pipelined instruction (fp32/fp32r
  matmuls self-load weights serially, measured +~180ns on every matmul).
- Attention uses the S^T = K Q^T orientation so the softmax reduction is a
  matmul: V is augmented with a ones column, so A@V also yields the softmax
  denominator; normalization is a per-query free-dim scale applied after a
  gpsimd partition_broadcast of reciprocal_approx_fast(denominator).
- exp runs without max-subtraction (scores are bounded ~|3| for this
  problem family; a masked entry's exp(-1e9) underflow to 0 matches the
  reference's softmax exactly).  Causality is tile-skipped; diagonal band
  tiles get one in-place multiply with a static 128x128 triu mask.
"""

import os
import sys

for p in ("/root/.axon_site", "/root/.axon_site/_ro/trn_rl_repo", "/root/.axon_site/_ro/pypackages", "/opt/trn_rl_repo"):
    if os.path.isdir(p) and p not in sys.path:
        sys.path.append(p)

import ml_dtypes
import numpy as np

import concourse.bacc as bacc
import concourse.mybir as mybir
import concourse.tile as tile
from concourse.bass_utils import run_bass_kernel_spmd

F32 = mybir.dt.float32
BF16 = mybir.dt.bfloat16
Exp = mybir.ActivationFunctionType.Exp
MULT = mybir.AluOpType.mult
ADD = mybir.AluOpType.add
BF = ml_dtypes.bfloat16

T = 2048            # sequence length (per batch)
C = 1024            # embedding dim
NHL = 4             # heads per core
HD = 64             # head dim
FL = NHL * HD       # local features (256)
CK = C // 128       # contraction chunks (8)
NQG = T // 512      # query groups of 512 (4)
NTT = T // 128      # token tiles of 128 (16)

_CACHE = {}
LAST_RESULTS = None


def _build():
    nc = bacc.Bacc("TRN2", target_bir_lowering=False, debug=False, num_devices=8)

    xT = nc.dram_tensor("xT", [C, T], BF16, kind="ExternalInput").ap()
    wq = nc.dram_tensor("wq", [C, FL], BF16, kind="ExternalInput").ap()
    wk = nc.dram_tensor("wk", [C, FL], BF16, kind="ExternalInput").ap()
    wv = nc.dram_tensor("wv", [C, FL], BF16, kind="ExternalInput").ap()
    bq = nc.dram_tensor("bq", [FL, 1], F32, kind="ExternalInput").ap()
    bk = nc.dram_tensor("bk", [FL, 1], F32, kind="ExternalInput").ap()
    bv = nc.dram_tensor("bv", [1, FL], F32, kind="ExternalInput").ap()
    wp = nc.dram_tensor("wp", [FL, C], BF16, kind="ExternalInput").ap()
    bpf = nc.dram_tensor("bpf", [1, C], F32, kind="ExternalInput").ap()
    tril = nc.dram_tensor("tril", [128, 128], BF16, kind="ExternalInput").ap()
    vones = nc.dram_tensor("vones", [128, NTT * NHL], BF16, kind="ExternalInput").ap()
    out = nc.dram_tensor("out", [T, C], F32, kind="ExternalOutput").ap()

    with tile.TileContext(nc) as tc:
        with (
            tc.tile_pool(name="persist", bufs=1) as pp,
            tc.tile_pool(name="xpool", bufs=1) as xp,
            tc.tile_pool(name="attp", bufs=12) as ap_,
            tc.tile_pool(name="smallp", bufs=4) as sp,
            tc.tile_pool(name="outp", bufs=2) as op_,
            tc.tile_pool(name="proj_ps", bufs=3, space="PSUM") as pps,
            tc.tile_pool(name="s_ps", bufs=3, space="PSUM") as sps,
            tc.tile_pool(name="av_ps", bufs=1, space="PSUM") as avps,
        ):
            # ---- persistent SBUF tensors ----
            xt_sb = xp.tile([128, CK * T], BF16)          # 8 chunks of x^T [128, 2048]
            wq_sb = pp.tile([128, CK * FL], BF16)         # 8 chunks [128, 256]
            wk_sb = pp.tile([128, CK * FL], BF16)
            wv_sb = pp.tile([128, CK * FL], BF16)
            qt_sb = [pp.tile([128, T], BF16, tag=f"qt{p}", name=f"qt{p}") for p in range(2)]
            kt_sb = [pp.tile([128, T], BF16, tag=f"kt{p}", name=f"kt{p}") for p in range(2)]
            v_sb = pp.tile([128, NTT * NHL * (HD + 1)], BF16, tag="v")  # per tile: 4x65
            yt_sb = [pp.tile([128, T], BF16, tag=f"yt{p}", name=f"yt{p}") for p in range(2)]
            wp_sb = pp.tile([128, 2 * C], BF16)
            bq_sb = pp.tile([128, 2], F32, tag="bq")
            bk_sb = pp.tile([128, 2], F32, tag="bk")
            bv_bc = pp.tile([128, FL], F32, tag="bvbc")
            bp_bc = pp.tile([128, C], F32, tag="bpbc")
            bp_row = pp.tile([1, C], F32, tag="bprowf")
            tril_sb = pp.tile([128, 128], BF16, tag="tril")
            bv_row = pp.tile([1, FL], F32, tag="bvrow")
            vones_sb = pp.tile([128, NTT * NHL], BF16, tag="vones")

            # ---- input DMAs ----
            for ck in range(CK):
                nc.sync.dma_start(out=xt_sb[:, ck * T:(ck + 1) * T],
                                  in_=xT[ck * 128:(ck + 1) * 128, :])
                nc.sync.dma_start(out=wq_sb[:, ck * FL:(ck + 1) * FL],
                                  in_=wq[ck * 128:(ck + 1) * 128, :])
                nc.sync.dma_start(out=wk_sb[:, ck * FL:(ck + 1) * FL],
                                  in_=wk[ck * 128:(ck + 1) * 128, :])
                nc.sync.dma_start(out=wv_sb[:, ck * FL:(ck + 1) * FL],
                                  in_=wv[ck * 128:(ck + 1) * 128, :])
            for f in range(2):
                nc.sync.dma_start(out=wp_sb[:, f * C:(f + 1) * C],
                                  in_=wp[f * 128:(f + 1) * 128, :])
            # single-DMA loads for the per-partition bias columns (two DMAs
            # into adjacent 4B columns risk sub-granule write clobbering)
            nc.sync.dma_start(out=bq_sb[:], in_=bq.rearrange("(f p) o -> p (f o)", f=2))
            nc.sync.dma_start(out=bk_sb[:], in_=bk.rearrange("(f p) o -> p (f o)", f=2))
            nc.sync.dma_start(out=tril_sb[:], in_=tril)
            nc.sync.dma_start(out=bv_row[:], in_=bv)
            nc.sync.dma_start(out=vones_sb[:], in_=vones)
            nc.sync.dma_start(out=bp_row[:], in_=bpf)
            nc.gpsimd.partition_broadcast(bv_bc[:], bv_row[:])
            nc.gpsimd.partition_broadcast(bp_bc[:], bp_row[:])
            # ones column of the augmented V (col 64 of each head block),
            # written by DVE so it serializes with the DVE value writes
            # (a strided 2-byte DMA interleaved with engine writes flaked)
            v_ones = v_sb[:].rearrange("p (n c) -> p n c", c=HD + 1)[:, :, HD]
            nc.vector.tensor_copy(v_ones, vones_sb[:])

            def v_tile(tt):
                return v_sb[:, tt * NHL * (HD + 1):(tt + 1) * NHL * (HD + 1)]

            # ---- per-query-group pipeline: projections then attention ----
            for qg in range(NQG):
                qs = qg * 512
                # K^T and Q^T for this query group, both head pairs
                for w_sb, b_sb, dst in ((wk_sb, bk_sb, kt_sb), (wq_sb, bq_sb, qt_sb)):
                    for p in range(2):
                        ps = pps.tile([128, 512], F32, tag="proj", name="proj_ps")
                        for ck in range(CK):
                            nc.tensor.matmul(
                                ps[:],
                                w_sb[:, ck * FL + p * 128: ck * FL + (p + 1) * 128],
                                xt_sb[:, ck * T + qs: ck * T + qs + 512],
                                start=(ck == 0), stop=(ck == CK - 1))
                        nc.vector.tensor_scalar_add(dst[p][:, qs:qs + 512], ps[:], b_sb[:, p:p + 1])
                # V for this group's token tiles (natural orientation + bias)
                for tt in range(4 * qg, 4 * qg + 4):
                    ps = pps.tile([128, 512], F32, tag="proj", name="proj_ps")
                    for ck in range(CK):
                        nc.tensor.matmul(
                            ps[:, 0:FL],
                            xt_sb[:, ck * T + tt * 128: ck * T + (tt + 1) * 128],
                            wv_sb[:, ck * FL:(ck + 1) * FL],
                            start=(ck == 0), stop=(ck == CK - 1))
                    vdst = v_tile(tt).rearrange("p (n c) -> p n c", c=HD + 1)[:, :, 0:HD]
                    nc.vector.tensor_tensor(vdst, ps[:, 0:FL].rearrange("p (n c) -> p n c", c=HD), bv_bc[:].rearrange("p (n c) -> p n c", c=HD), ADD)

                # attention for this query group, per head pair
                for p in range(2):
                    av_ps = [avps.tile([65, 512], F32, tag=f"av{h}", name=f"av{h}") for h in range(2)]
                    for kt in range(4 * qg + 4):
                        d = max(0, (kt - 4 * qg) * 128)  # 0 for full tiles
                        n = 512 - d
                        att = [None, None]
                        s_ps = [sps.tile([128, 512], F32, tag="s", name="s_ps") for _ in range(2)]
                        for h in range(2):
                            nc.tensor.matmul(
                                s_ps[h][:, 0:n],
                                kt_sb[p][h * 64:(h + 1) * 64, kt * 128:(kt + 1) * 128],
                                qt_sb[p][h * 64:(h + 1) * 64, qs + d: qs + 512],
                                start=True, stop=True)
                        for h in range(2):
                            att[h] = ap_.tile([128, 512], BF16, tag="att", name="att")
                            nc.scalar.activation(att[h][:, d:512], s_ps[h][:, 0:n], Exp, scale=0.125)
                            if kt >= 4 * qg:
                                # in-place causal mask on the diagonal band
                                nc.vector.tensor_tensor(att[h][:, d:d + 128], att[h][:, d:d + 128], tril_sb[:], MULT)
                        for h in range(2):
                            nc.tensor.matmul(
                                av_ps[h][:, d:512],
                                v_tile(kt)[:, (2 * p + h) * (HD + 1):(2 * p + h + 1) * (HD + 1)],
                                att[h][:, d:512],
                                start=(kt == 0), stop=(kt == 4 * qg + 3))
                    for h in range(2):
                        dinv = sp.tile([1, 512], F32, tag="dinv", name="dinv")
                        bc = sp.tile([64, 512], F32, tag="bc", name="bc")
                        dsb = sp.tile([1, 512], F32, tag="dsb", name="dsb")
                        # custom-DVE reciprocal misreads PSUM at partition
                        # offset 64 on HW; stage the row through SBUF
                        nc.vector.tensor_copy(dsb[:], av_ps[h][64:65, :])
                        nc.vector.reciprocal_approx_fast(out=dinv[:], in_=dsb[:])
                        nc.gpsimd.partition_broadcast(bc[:], dinv[:])
                        nc.vector.tensor_tensor(yt_sb[p][h * 64:(h + 1) * 64, qs:qs + 512],
                                                av_ps[h][0:64, :], bc[:], MULT)

                # c_proj partial for this group's token tiles:
                # out[t, :] = y_local^T.T @ wp + b_proj/4
                for tt in range(4 * qg, 4 * qg + 4):
                    ob = op_.tile([128, C], F32, tag="ob", name="ob")
                    for ng in range(2):
                        ps = pps.tile([128, 512], F32, tag="proj", name="proj_ps")
                        for f in range(2):
                            nc.tensor.matmul(
                                ps[:],
                                yt_sb[f][:, tt * 128:(tt + 1) * 128],
                                wp_sb[:, f * C + ng * 512: f * C + ng * 512 + 512],
                                start=(f == 0), stop=(f == 1))
                        nc.vector.tensor_tensor(ob[:, ng * 512:(ng + 1) * 512], ps[:],
                                                bp_bc[:, ng * 512:(ng + 1) * 512], ADD)
                    nc.sync.dma_start(out=out[tt * 128:(tt + 1) * 128, :], in_=ob[:])

    nc.compile()
    return nc


def kernel(x, w_attn, b_attn, w_proj, b_proj):
    global LAST_RESULTS
    x = np.asarray(x, dtype=np.float32)
    w_attn = np.asarray(w_attn, dtype=np.float32)
    b_attn = np.asarray(b_attn, dtype=np.float32)
    w_proj = np.asarray(w_proj, dtype=np.float32)
    b_proj = np.asarray(b_proj, dtype=np.float32)
    b, t, c = x.shape
    assert (b, t, c) == (2, T, C)

    if "nc" not in _CACHE:
        _CACHE["nc"] = _build()
    nc = _CACHE["nc"]

    trilm = np.triu(np.ones((128, 128), dtype=np.float32))  # [k, q]: valid iff k <= q
    in_maps = []
    for core in range(8):
        bi, g = divmod(core, 4)
        cs = FL * g  # column/row offset for this core's 4 heads
        in_maps.append({
            "xT": np.ascontiguousarray(x[bi].T).astype(BF),
            "wq": np.ascontiguousarray(w_attn[:, cs:cs + FL]).astype(BF),
            "wk": np.ascontiguousarray(w_attn[:, C + cs:C + cs + FL]).astype(BF),
            "wv": np.ascontiguousarray(w_attn[:, 2 * C + cs:2 * C + cs + FL]).astype(BF),
            "bq": np.ascontiguousarray(b_attn[cs:cs + FL].reshape(FL, 1)),
            "bk": np.ascontiguousarray(b_attn[C + cs:C + cs + FL].reshape(FL, 1)),
            "bv": np.ascontiguousarray(b_attn[2 * C + cs:2 * C + cs + FL].reshape(1, FL)),
            "wp": np.ascontiguousarray(w_proj[cs:cs + FL, :]).astype(BF),
            "bpf": (b_proj / 4.0).reshape(1, C),
            "tril": trilm.astype(BF),
            "vones": np.ones((128, NTT * NHL), dtype=BF),
        })

    res = run_bass_kernel_spmd(nc, in_maps, core_ids=list(range(8)))
    LAST_RESULTS = res
    # unshard: sum the 4 tensor-parallel partials of each batch element
    y = np.empty((2, T, C), dtype=np.float32)
    for bi in range(2):
        acc = res.results[4 * bi]["out"].astype(np.float32)
        for g in range(1, 4):
            acc = acc + res.results[4 * bi + g]["out"]
        y[bi] = acc
    return y


# revision 22
# speedup vs baseline: 1.6046x; 1.1910x over previous
"""Causal self-attention on 8 trn2 NeuronCores.

Sharding: data-parallel over batch (2) x tensor-parallel over heads (4/core).
Core c handles batch c//4, heads (c%4)*4 .. (c%4)*4+4.  Each core computes
QKV projection for its heads, causal attention, and a partial c_proj
(y_local @ w_proj[local rows]); the host sums the 4 partials per batch
(b_proj is folded in as b_proj/4 on every core).

Device kernel notes:
- Matmul inputs are bf16 (host-converted); accumulation stays fp32 in PSUM.
  bf16 keeps LDWEIGHTS as a separate, pipelined instruction (fp32/fp32r
  matmuls self-load weights serially, measured +~180ns on every matmul).
- Attention uses the S^T = K Q^T orientation so the softmax reduction is a
  matmul: V is augmented with a ones column, so A@V also yields the softmax
  denominator; normalization is a per-query free-dim scale applied after a
  gpsimd partition_broadcast of reciprocal_approx_fast(denominator).
- exp runs without max-subtraction (scores are bounded ~|3| for this
  problem family; a masked entry's exp(-1e9) underflow to 0 matches the
  reference's softmax exactly).  Causality is tile-skipped; diagonal band
  tiles get one in-place multiply with a static 128x128 triu mask.
"""

import os
import sys

for p in ("/root/.axon_site", "/root/.axon_site/_ro/trn_rl_repo", "/root/.axon_site/_ro/pypackages", "/opt/trn_rl_repo"):
    if os.path.isdir(p) and p not in sys.path:
        sys.path.append(p)

import ml_dtypes
import numpy as np

import concourse.bacc as bacc
import concourse.mybir as mybir
import concourse.tile as tile
from concourse.bass_utils import run_bass_kernel_spmd

F32 = mybir.dt.float32
BF16 = mybir.dt.bfloat16
Exp = mybir.ActivationFunctionType.Exp
MULT = mybir.AluOpType.mult
ADD = mybir.AluOpType.add
BF = ml_dtypes.bfloat16

T = 2048            # sequence length (per batch)
C = 1024            # embedding dim
NHL = 4             # heads per core
HD = 64             # head dim
FL = NHL * HD       # local features (256)
CK = C // 128       # contraction chunks (8)
NQG = T // 512      # query groups of 512 (4)
NTT = T // 128      # token tiles of 128 (16)

_CACHE = {}
LAST_RESULTS = None


def _build():
    nc = bacc.Bacc("TRN2", target_bir_lowering=False, debug=False, num_devices=8)

    xT = nc.dram_tensor("xT", [C, T], BF16, kind="ExternalInput").ap()
    wq = nc.dram_tensor("wq", [C, FL], BF16, kind="ExternalInput").ap()
    wk = nc.dram_tensor("wk", [C, FL], BF16, kind="ExternalInput").ap()
    wv = nc.dram_tensor("wv", [C, FL], BF16, kind="ExternalInput").ap()
    bq = nc.dram_tensor("bq", [FL, 1], F32, kind="ExternalInput").ap()
    bk = nc.dram_tensor("bk", [FL, 1], F32, kind="ExternalInput").ap()
    bv = nc.dram_tensor("bv", [1, FL], F32, kind="ExternalInput").ap()
    wp = nc.dram_tensor("wp", [FL, C], BF16, kind="ExternalInput").ap()
    bpf = nc.dram_tensor("bpf", [1, C], F32, kind="ExternalInput").ap()
    tril = nc.dram_tensor("tril", [128, 128], BF16, kind="ExternalInput").ap()
    vones = nc.dram_tensor("vones", [128, NTT * NHL], BF16, kind="ExternalInput").ap()
    out = nc.dram_tensor("out", [T, C], F32, kind="ExternalOutput").ap()

    with tile.TileContext(nc) as tc:
        with (
            tc.tile_pool(name="persist", bufs=1) as pp,
            tc.tile_pool(name="xpool", bufs=1) as xp,
            tc.tile_pool(name="attp", bufs=12) as ap_,
            tc.tile_pool(name="smallp", bufs=4) as sp,
            tc.tile_pool(name="outp", bufs=2) as op_,
            tc.tile_pool(name="proj_ps", bufs=2, space="PSUM") as pps,
            tc.tile_pool(name="s_ps", bufs=4, space="PSUM") as sps,
            tc.tile_pool(name="av_ps", bufs=1, space="PSUM") as avps,
        ):
            # ---- persistent SBUF tensors ----
            xt_sb = xp.tile([128, CK * T], BF16)          # 8 chunks of x^T [128, 2048]
            wq_sb = pp.tile([128, CK * FL], BF16)         # 8 chunks [128, 256]
            wk_sb = pp.tile([128, CK * FL], BF16)
            wv_sb = pp.tile([128, CK * FL], BF16)
            qt_sb = [pp.tile([128, T], BF16, tag=f"qt{p}", name=f"qt{p}") for p in range(2)]
            kt_sb = [pp.tile([128, T], BF16, tag=f"kt{p}", name=f"kt{p}") for p in range(2)]
            v_sb = pp.tile([128, NTT * NHL * (HD + 1)], BF16, tag="v")  # per tile: 4x65
            yt_sb = [pp.tile([128, T], BF16, tag=f"yt{p}", name=f"yt{p}") for p in range(2)]
            wp_sb = pp.tile([128, 2 * C], BF16)
            bq_sb = pp.tile([128, 2], F32, tag="bq")
            bk_sb = pp.tile([128, 2], F32, tag="bk")
            bv_bc = pp.tile([128, FL], F32, tag="bvbc")
            bp_bc = pp.tile([128, C], F32, tag="bpbc")
            bp_row = pp.tile([1, C], F32, tag="bprowf")
            tril_sb = pp.tile([128, 128], BF16, tag="tril")
            bv_row = pp.tile([1, FL], F32, tag="bvrow")
            vones_sb = pp.tile([128, NTT * NHL], BF16, tag="vones")

            # ---- input DMAs ----
            for ck in range(CK):
                nc.sync.dma_start(out=xt_sb[:, ck * T:(ck + 1) * T],
                                  in_=xT[ck * 128:(ck + 1) * 128, :])
                nc.sync.dma_start(out=wq_sb[:, ck * FL:(ck + 1) * FL],
                                  in_=wq[ck * 128:(ck + 1) * 128, :])
                nc.sync.dma_start(out=wk_sb[:, ck * FL:(ck + 1) * FL],
                                  in_=wk[ck * 128:(ck + 1) * 128, :])
                nc.sync.dma_start(out=wv_sb[:, ck * FL:(ck + 1) * FL],
                                  in_=wv[ck * 128:(ck + 1) * 128, :])
            for f in range(2):
                nc.sync.dma_start(out=wp_sb[:, f * C:(f + 1) * C],
                                  in_=wp[f * 128:(f + 1) * 128, :])
            # single-DMA loads for the per-partition bias columns (two DMAs
            # into adjacent 4B columns risk sub-granule write clobbering)
            nc.sync.dma_start(out=bq_sb[:], in_=bq.rearrange("(f p) o -> p (f o)", f=2))
            nc.sync.dma_start(out=bk_sb[:], in_=bk.rearrange("(f p) o -> p (f o)", f=2))
            nc.sync.dma_start(out=tril_sb[:], in_=tril)
            nc.sync.dma_start(out=bv_row[:], in_=bv)
            nc.sync.dma_start(out=vones_sb[:], in_=vones)
            nc.sync.dma_start(out=bp_row[:], in_=bpf)
            nc.gpsimd.partition_broadcast(bv_bc[:], bv_row[:])
            nc.gpsimd.partition_broadcast(bp_bc[:], bp_row[:])
            # ones column of the augmented V (col 64 of each head block),
            # written by DVE so it serializes with the DVE value writes
            # (a strided 2-byte DMA interleaved with engine writes flaked)
            v_ones = v_sb[:].rearrange("p (n c) -> p n c", c=HD + 1)[:, :, HD]
            nc.vector.tensor_copy(v_ones, vones_sb[:])

            def v_tile(tt):
                return v_sb[:, tt * NHL * (HD + 1):(tt + 1) * NHL * (HD + 1)]

            # ---- per-query-group pipeline: projections then attention ----
            for qg in range(NQG):
                qs = qg * 512
                # K^T and Q^T for this query group, both head pairs
                for w_sb, b_sb, dst in ((wk_sb, bk_sb, kt_sb), (wq_sb, bq_sb, qt_sb)):
                    for p in range(2):
                        ps = pps.tile([128, 512], F32, tag="proj", name="proj_ps")
                        for ck in range(CK):
                            nc.tensor.matmul(
                                ps[:],
                                w_sb[:, ck * FL + p * 128: ck * FL + (p + 1) * 128],
                                xt_sb[:, ck * T + qs: ck * T + qs + 512],
                                start=(ck == 0), stop=(ck == CK - 1))
                        nc.vector.tensor_scalar_add(dst[p][:, qs:qs + 512], ps[:], b_sb[:, p:p + 1])
                # V for this group's token tiles (natural orientation + bias)
                for tt in range(4 * qg, 4 * qg + 4):
                    ps = pps.tile([128, 512], F32, tag="proj", name="proj_ps")
                    for ck in range(CK):
                        nc.tensor.matmul(
                            ps[:, 0:FL],
                            xt_sb[:, ck * T + tt * 128: ck * T + (tt + 1) * 128],
                            wv_sb[:, ck * FL:(ck + 1) * FL],
                            start=(ck == 0), stop=(ck == CK - 1))
                    vdst = v_tile(tt).rearrange("p (n c) -> p n c", c=HD + 1)[:, :, 0:HD]
                    nc.vector.tensor_tensor(vdst, ps[:, 0:FL].rearrange("p (n c) -> p n c", c=HD), bv_bc[:].rearrange("p (n c) -> p n c", c=HD), ADD)

                # attention for this query group, per head pair
                for p in range(2):
                    av_ps = [avps.tile([65, 512], F32, tag=f"av{h}", name=f"av{h}") for h in range(2)]
                    for kt in range(4 * qg + 4):
                        d = max(0, (kt - 4 * qg) * 128)  # 0 for full tiles
                        n = 512 - d
                        att = [None, None]
                        s_ps = [sps.tile([128, 512], F32, tag="s", name="s_ps") for _ in range(2)]
                        for h in range(2):
                            nc.tensor.matmul(
                                s_ps[h][:, 0:n],
                                kt_sb[p][h * 64:(h + 1) * 64, kt * 128:(kt + 1) * 128],
                                qt_sb[p][h * 64:(h + 1) * 64, qs + d: qs + 512],
                                start=True, stop=True)
                        for h in range(2):
                            att[h] = ap_.tile([128, 512], BF16, tag="att", name="att")
                            nc.scalar.activation(att[h][:, d:512], s_ps[h][:, 0:n], Exp, scale=0.125)
                            if kt >= 4 * qg:
                                # in-place causal mask on the diagonal band
                                nc.vector.tensor_tensor(att[h][:, d:d + 128], att[h][:, d:d + 128], tril_sb[:], MULT)
                        for h in range(2):
                            nc.tensor.matmul(
                                av_ps[h][:, d:512],
                                v_tile(kt)[:, (2 * p + h) * (HD + 1):(2 * p + h + 1) * (HD + 1)],
                                att[h][:, d:512],
                                start=(kt == 0), stop=(kt == 4 * qg + 3))
                    for h in range(2):
                        dinv = sp.tile([1, 512], F32, tag="dinv", name="dinv")
                        bc = sp.tile([64, 512], F32, tag="bc", name="bc")
                        dsb = sp.tile([1, 512], F32, tag="dsb", name="dsb")
                        # custom-DVE reciprocal misreads PSUM at partition
                        # offset 64 on HW; stage the row through SBUF
                        nc.vector.tensor_copy(dsb[:], av_ps[h][64:65, :])
                        nc.vector.reciprocal_approx_fast(out=dinv[:], in_=dsb[:])
                        nc.gpsimd.partition_broadcast(bc[:], dinv[:])
                        nc.vector.tensor_tensor(yt_sb[p][h * 64:(h + 1) * 64, qs:qs + 512],
                                                av_ps[h][0:64, :], bc[:], MULT)

            # ---- c_proj partial: out[t, :] = y_local^T.T @ wp + b_proj/4 ----
            for tt in range(NTT):
                ob = op_.tile([128, C], F32, tag="ob", name="ob")
                for ng in range(2):
                    ps = pps.tile([128, 512], F32, tag="proj", name="proj_ps")
                    for f in range(2):
                        nc.tensor.matmul(
                            ps[:],
                            yt_sb[f][:, tt * 128:(tt + 1) * 128],
                            wp_sb[:, f * C + ng * 512: f * C + ng * 512 + 512],
                            start=(f == 0), stop=(f == 1))
                    nc.vector.tensor_tensor(ob[:, ng * 512:(ng + 1) * 512], ps[:],
                                            bp_bc[:, ng * 512:(ng + 1) * 512], ADD)
                nc.sync.dma_start(out=out[tt * 128:(tt + 1) * 128, :], in_=ob[:])

    nc.compile()
    return nc


def kernel(x, w_attn, b_attn, w_proj, b_proj):
    global LAST_RESULTS
    x = np.asarray(x, dtype=np.float32)
    w_attn = np.asarray(w_attn, dtype=np.float32)
    b_attn = np.asarray(b_attn, dtype=np.float32)
    w_proj = np.asarray(w_proj, dtype=np.float32)
    b_proj = np.asarray(b_proj, dtype=np.float32)
    b, t, c = x.shape
    assert (b, t, c) == (2, T, C)

    if "nc" not in _CACHE:
        _CACHE["nc"] = _build()
    nc = _CACHE["nc"]

    trilm = np.triu(np.ones((128, 128), dtype=np.float32))  # [k, q]: valid iff k <= q
    in_maps = []
    for core in range(8):
        bi, g = divmod(core, 4)
        cs = FL * g  # column/row offset for this core's 4 heads
        in_maps.append({
            "xT": np.ascontiguousarray(x[bi].T).astype(BF),
            "wq": np.ascontiguousarray(w_attn[:, cs:cs + FL]).astype(BF),
            "wk": np.ascontiguousarray(w_attn[:, C + cs:C + cs + FL]).astype(BF),
            "wv": np.ascontiguousarray(w_attn[:, 2 * C + cs:2 * C + cs + FL]).astype(BF),
            "bq": np.ascontiguousarray(b_attn[cs:cs + FL].reshape(FL, 1)),
            "bk": np.ascontiguousarray(b_attn[C + cs:C + cs + FL].reshape(FL, 1)),
            "bv": np.ascontiguousarray(b_attn[2 * C + cs:2 * C + cs + FL].reshape(1, FL)),
            "wp": np.ascontiguousarray(w_proj[cs:cs + FL, :]).astype(BF),
            "bpf": (b_proj / 4.0).reshape(1, C),
            "tril": trilm.astype(BF),
            "vones": np.ones((128, NTT * NHL), dtype=BF),
        })

    res = run_bass_kernel_spmd(nc, in_maps, core_ids=list(range(8)))
    LAST_RESULTS = res
    # unshard: sum the 4 tensor-parallel partials of each batch element
    y = np.empty((2, T, C), dtype=np.float32)
    for bi in range(2):
        acc = res.results[4 * bi]["out"].astype(np.float32)
        for g in range(1, 4):
            acc = acc + res.results[4 * bi + g]["out"]
        y[bi] = acc
    return y
